# revision 4
# baseline (speedup 1.0000x reference)
"""Trainium2 Bass kernel for CAGKE (Gaussian-kernel spike embedding).

Math: psedu[t] = sum_d softmax(weight)[d] * (spikes (*) K_d)[t] + noise[t],
followed by global min-max normalization. The softmax weights do not depend
on t, so the weighted sum over the D=128 kernel bank commutes with the
convolution: psedu = spikes (*) kbar + noise, where
kbar(delta) = sum_d sw_d * (C/sigma_d) * exp(-(delta-1)^2 / (2 sigma_d^2)).

The Gaussian bank (sigma <= 3.0) underflows f32 beyond |delta-1| ~ 44, so an
89-tap kbar window inside a 511-tap zero-padded DRAM buffer reproduces the
reference conv bit-tight. The conv is 3 banded 128-contraction matmuls with
the partition-flipped spikes stationary and the kbar Toeplitz bank moving
(psedu_rm[c, p] = sum_qt spf[qt, c+b] * L_k[qt, p]); the Toeplitz bank comes
from a DRAM bounce (overlapping-window reads are only well-defined on the
DRAM side of a DMA, and the BIR verifier only allows ascending outer
strides, hence the flipped contraction).

Latency-focused layout (the whole kernel is one serial dependency chain;
every DMA leg costs ~2us of issue+completion latency):
 - sigma and weight are fetched as single-descriptor [1,128] rows into two
   partitions of one tile and PE-transposed to [128, 2], instead of a
   128-descriptor per-element spray (saves ~0.5us on the critical path).
 - the softmax numerator exp(w_d) is applied per-partition: the contraction
   lhsT is v_d = C * exp(w_d) / sigma_d, so the baseline's weight-broadcast
   matmul and accumulator read disappear.  An extra rhs column sigma_d/C
   makes the same matmul emit esum = sum_d exp(w_d), which scales the noise
   (min-max normalization forgives the global esum factor).
 - all three Toeplitz chunks ride the two HWDGE rings (sync + scalar); the
   SWDGE/gpsimd ring is ~1.5us slower per DMA and is not used at all.
 - the conv contracts in bf16 (1 cycle/row vs 4 for double-pumped fp32,
   and half the readback bytes). kbar itself is computed in f32 and cast
   once at the PSUM->SBUF copy, so each tap takes a single unbiased bf16
   rounding (~4e-3 worst-case) and the spikes are 0/1, exact in bf16; the
   resulting output error ~1e-3 sits far inside the 2e-2 gate.

All 8 cores run the identical replicated program (total I/O is ~100KB, far
below the point where sharding would beat collective/sync overhead); the
host takes core 0's output.
"""

import os
import sys

for _p in ("/opt/trn_rl_repo", "/root/.axon_site/_ro/trn_rl_repo"):
    if os.path.isdir(_p) and _p not in sys.path:
        sys.path.insert(0, _p)

import numpy as np

T = 8192  # in_length
D = 128  # embed_dim (kernel bank size)
GAUSS_C = 0.39894228  # 1/sqrt(2*pi) as hardcoded in the source module
NCORES = 8
COLS = T // 128  # 64 columns of 128 contiguous time steps
KW = 511  # kbar taps, delta in [-255, 255]
J0, JW = 212, 89  # nonzero kbar window: j in [212, 301) -> delta in [-44, 45)

_CACHE = {}


def _build_bass():
    import concourse.bass as bass
    import concourse.tile as tile
    from concourse import bacc, mybir
    from concourse.bass import _add_dep_helper as add_dep

    f32 = mybir.dt.float32
    bf16 = mybir.dt.bfloat16
    nc = bacc.Bacc("TRN2", target_bir_lowering=False, debug=False, num_devices=NCORES)

    x_d = nc.dram_tensor("X", [1, T], f32, kind="ExternalInput")
    w_d = nc.dram_tensor("weight", [1, D], f32, kind="ExternalInput")
    n_d = nc.dram_tensor("noise", [1, T], f32, kind="ExternalInput")
    s_d = nc.dram_tensor("sigma", [D], f32, kind="ExternalInput")
    o_d = nc.dram_tensor("out", [1, T], f32, kind="ExternalOutput")

    kb_d = nc.dram_tensor("kb_scratch", [KW], bf16)  # internal DRAM bounce

    with tile.TileContext(nc) as tc:
        with (
            tc.tile_pool(name="sb", bufs=1) as sb,
            tc.tile_pool(name="ps", bufs=1, space="PSUM") as ps,
        ):
            # ---- input DMAs, most-critical first; two HWDGE rings ----
            # sync(SP) ring: sigma first (heads the kbar chain), then X,
            # then the kbar-buffer edge zeros.
            sw2 = sb.tile([2, 128], f32)  # row 0 = sigma, row 1 = weight
            nc.sync.dma_start(out=sw2[0:1, :], in_=s_d.ap().unsqueeze(0))
            # scalar(ACT) ring: weight first (also heads the chain), noise.
            nc.scalar.dma_start(out=sw2[1:2, :], in_=w_d.ap())
            m_x = sb.tile([COLS, 128], f32)
            nc.sync.dma_start(
                out=m_x[:], in_=x_d.ap().rearrange("a (c p) -> (a c) p", p=128)
            )
            nrm = sb.tile([COLS, 128], f32)
            nc.scalar.dma_start(
                out=nrm[:], in_=n_d.ap().rearrange("a (c p) -> (a c) p", p=128)
            )
            zer = sb.tile([1, KW - JW], bf16)
            nc.vector.memset(zer[:], 0.0)
            nc.sync.dma_start(out=kb_d.ap()[0:J0].unsqueeze(0), in_=zer[:, 0:J0])
            nc.sync.dma_start(
                out=kb_d.ap()[J0 + JW : KW].unsqueeze(0),
                in_=zer[:, 0 : KW - J0 - JW],
            )

            # ---- gpsimd constants in criticality order: iota feeds dsq;
            # id64's top-left corner is the [2,2] transpose identity ----
            jj = sb.tile([D, JW], f32)  # j - 256, exact in f32
            nc.gpsimd.iota(
                jj[:], pattern=[[1, JW]], base=J0 - 256, channel_multiplier=0,
                allow_small_or_imprecise_dtypes=True,
            )
            id64 = sb.tile([COLS, COLS], f32)
            nc.gpsimd.memset(id64[:], 0.0)
            nc.gpsimd.affine_select(
                out=id64[:], in_=id64[:], compare_op=mybir.AluOpType.not_equal,
                fill=1.0, base=0, pattern=[[-1, COLS]], channel_multiplier=1,
            )
            jx128 = sb.tile([128, 128], f32)  # exchange matrix (anti-diagonal)
            nc.gpsimd.memset(jx128[:], 0.0)
            nc.gpsimd.affine_select(
                out=jx128[:], in_=jx128[:], compare_op=mybir.AluOpType.not_equal,
                fill=1.0, base=-127, pattern=[[1, 128]], channel_multiplier=1,
            )
            ones2 = sb.tile([2, COLS], f32)
            nc.gpsimd.memset(ones2[:], 1.0)
            # mconst = [[1, 0], [1, -1]]: one matmul then maps
            # g = [gmax, -gmin] to [range, gmin] broadcast over partitions
            mconst = sb.tile([2, 2], f32)
            nc.gpsimd.memset(mconst[:], 1.0)
            nc.gpsimd.affine_select(
                out=mconst[:], in_=mconst[:], compare_op=mybir.AluOpType.not_equal,
                fill=0.0, base=-1, pattern=[[1, 2]], channel_multiplier=2,
            )  # zero at (r0, f1)
            nc.gpsimd.affine_select(
                out=mconst[:], in_=mconst[:], compare_op=mybir.AluOpType.not_equal,
                fill=-1.0, base=-3, pattern=[[1, 2]], channel_multiplier=2,
            )  # -1 at (r1, f1)
            # dead halves of the Toeplitz chunks + spf halo columns
            l0 = sb.tile([128, 128], bf16, tag="L0")
            l1 = sb.tile([128, 128], bf16, tag="L1")
            l2 = sb.tile([128, 128], bf16, tag="L2")
            nc.gpsimd.memset(l0[0:64, :], 0.0)
            nc.gpsimd.memset(l2[64:128, :], 0.0)
            spf = sb.tile([128, COLS + 2], bf16)  # zero halo columns at 0 and 65
            nc.gpsimd.memset(spf[:, 0:1], 0.0)
            nc.gpsimd.memset(spf[:, COLS + 1 : COLS + 2], 0.0)

            dsq = sb.tile([D, JW], f32)
            nc.vector.tensor_mul(dsq[:], jj[:], jj[:])  # (j - 256)^2

            # ---- sigma/weight onto partitions via one PE transpose ----
            swt = ps.tile([128, 2], f32, tag="ps_a")
            i_swt = nc.tensor.transpose(swt[:], sw2[:], id64[0:2, 0:2])

            # the whole kbar chain runs at elevated scheduler priority: the
            # cost model does not see DMA completion latency, so it otherwise
            # lets spike-path work cut ahead on the in-order engines
            with tc.high_priority():
                inv_sig = sb.tile([D, 1], f32)
                i_recip = nc.vector.reciprocal(inv_sig[:], swt[:, 0:1])
                expm = sb.tile([D, JW + 1], f32)
                # col JW = sigma/C: the kb matmul's esum column
                nc.vector.tensor_scalar(
                    out=expm[:, JW : JW + 1], in0=swt[:, 0:1],
                    scalar1=1.0 / GAUSS_C, scalar2=None,
                    op0=mybir.AluOpType.mult,
                )
                nhalf = sb.tile([D, 1], f32)  # -1/(2 sigma^2)
                nc.vector.scalar_tensor_tensor(
                    out=nhalf[:], in0=inv_sig[:], scalar=-0.5, in1=inv_sig[:],
                    op0=mybir.AluOpType.mult, op1=mybir.AluOpType.mult,
                )
                exp_w = sb.tile([D, 1], f32)  # softmax numerator e_d
                i_expw = nc.scalar.activation(
                    out=exp_w[:], in_=swt[:, 1:2],
                    func=mybir.ActivationFunctionType.Exp, bias=0.0, scale=1.0,
                )
                i_expm = nc.scalar.activation(
                    out=expm[:, 0:JW], in_=dsq[:],
                    func=mybir.ActivationFunctionType.Exp,
                    bias=0.0, scale=nhalf[:, 0:1],
                )  # per-sigma gaussian row
                v = sb.tile([D, 1], f32)  # C * e_d / sigma_d
                nc.vector.scalar_tensor_tensor(
                    out=v[:], in0=exp_w[:], scalar=GAUSS_C, in1=inv_sig[:],
                    op0=mybir.AluOpType.mult, op1=mybir.AluOpType.mult,
                )

                # ---- kbar row + esum in one matmul ----
                kb_ps = ps.tile([1, JW + 1], f32, tag="ps_b")
                i_kbmm = nc.tensor.matmul(
                    kb_ps[:], lhsT=v[:], rhs=expm[:], start=True, stop=True,
                )
                kb_sb = sb.tile([1, JW], bf16)
                i_kbcopy = nc.vector.tensor_copy(kb_sb[:], kb_ps[:, 0:JW])
                nc.sync.dma_start(
                    out=kb_d.ap()[J0 : J0 + JW].unsqueeze(0), in_=kb_sb[:]
                )
                esum_sb = sb.tile([1, 1], f32)
                nc.vector.tensor_copy(esum_sb[:], kb_ps[:, JW : JW + 1])
                # L_k[qt, p] = kbar[qt + p + 128k] (flipped contraction
                # qt = 127-q; the spike operand is partition-flipped to
                # match). All-positive strides; chunks ride both HWDGE
                # rings. kbar is nonzero only on [J0, J0+JW), so band 0 is
                # zero for qt < 64 and band 2 for qt > 64: those halves are
                # memset early and only the live rows are fetched.
                nc.scalar.dma_start(
                    out=l0[64:128, :],
                    in_=bass.AP(
                        tensor=kb_d.ap().tensor, offset=64, ap=[[1, 64], [1, 128]]
                    ),
                )
                nc.sync.dma_start(
                    out=l1[:],
                    in_=bass.AP(
                        tensor=kb_d.ap().tensor, offset=128, ap=[[1, 128], [1, 128]]
                    ),
                )
                nc.scalar.dma_start(
                    out=l2[0:65, :],
                    in_=bass.AP(
                        tensor=kb_d.ap().tensor, offset=256, ap=[[1, 65], [1, 128]]
                    ),
                )

            # ---- spikes: threshold, PE transpose, partition flip ----
            spk = sb.tile([COLS, 128], f32)
            i_thr = nc.vector.tensor_scalar(
                out=spk[:], in0=m_x[:], scalar1=0.5, scalar2=None,
                op0=mybir.AluOpType.is_gt,
            )
            sp_ps = ps.tile([128, COLS], f32, tag="ps_c")
            i_spt = nc.tensor.transpose(sp_ps[:], spk[:], id64[:])
            sp_sb = sb.tile([128, COLS], f32)
            i_spcopy = nc.vector.tensor_copy(sp_sb[:], sp_ps[:])
            spf_ps = ps.tile([128, COLS], f32, tag="ps_d")
            i_spf = nc.tensor.matmul(
                spf_ps[:], lhsT=jx128[:], rhs=sp_sb[:], start=True, stop=True,
            )  # partition-flip: spf_ps[qt, c] = spikes[128c + 127 - qt]
            i_spfcopy = nc.vector.tensor_copy(spf[:, 1 : COLS + 1], spf_ps[:])
            # keep the in-order DVE and PE streams from letting spike-path
            # work delay the serial kbar chain
            add_dep(i_thr.ins, i_recip.ins, sync=False, reason="kbar chain first")
            add_dep(i_spcopy.ins, i_kbcopy.ins, sync=False, reason="kbar chain first")
            add_dep(i_spfcopy.ins, i_kbcopy.ins, sync=False, reason="kbar chain first")
            add_dep(i_spt.ins, i_kbmm.ins, sync=False, reason="kbar chain first")

            # ---- esum broadcast onto the 64 output partitions ----
            es_ps = ps.tile([COLS, 1], f32, tag="ps_e")
            i_esmm = nc.tensor.matmul(
                es_ps[:], lhsT=ones2[0:1, :], rhs=esum_sb[:],
                start=True, stop=True,
            )
            add_dep(i_esmm.ins, i_kbmm.ins, sync=False, reason="kbar chain first")

            # ---- banded conv, row-major output: spikes stationary, kbar
            # moving: psedu_rm[c, p] = sum_qt spf[qt, c+b] * L_k[qt, p] ----
            conv_ps = ps.tile([COLS, 128], f32, tag="ps_f")
            lchunks = [l0, l1, l2]
            for i, (k, b) in enumerate(((0, 1), (1, 0), (2, -1))):
                nc.tensor.matmul(
                    conv_ps[:],
                    lhsT=spf[:, 1 + b : COLS + 1 + b],
                    rhs=lchunks[k][:],
                    start=(i == 0),
                    stop=(i == 2),
                )

            # ---- add esum-scaled noise (still row-major) ----
            ps_rm = sb.tile([COLS, 128], f32)
            nc.vector.scalar_tensor_tensor(
                out=ps_rm[:], in0=nrm[:], scalar=es_ps[:, 0:1], in1=conv_ps[:],
                op0=mybir.AluOpType.mult, op1=mybir.AluOpType.add,
            )  # esum * (conv_true + noise_true) up to the global scale

            # ---- global min/max + normalize ----
            # Per-partition stats packed as [max, -min], PE-transposed to one
            # row pair, reduced along free; then gg = mconst * g and one
            # matmul against ones broadcasts [range, gmin] to all partitions.
            pk = sb.tile([COLS, 2], f32)
            nc.vector.tensor_reduce(
                out=pk[:, 0:1], in_=ps_rm[:], axis=mybir.AxisListType.X,
                op=mybir.AluOpType.max,
            )
            nc.vector.tensor_reduce(
                out=pk[:, 1:2], in_=ps_rm[:], axis=mybir.AxisListType.X,
                op=mybir.AluOpType.min, negate=True,
            )
            pk_ps = ps.tile([2, COLS], f32, tag="ps_c")
            nc.tensor.transpose(pk_ps[:], pk[:], id64[:])
            g = sb.tile([2, 1], f32)
            nc.vector.tensor_reduce(
                out=g[:], in_=pk_ps[:], axis=mybir.AxisListType.X,
                op=mybir.AluOpType.max,
            )  # g[0] = gmax, g[1] = -gmin
            gg = sb.tile([2, 2], f32)
            nc.vector.tensor_scalar_mul(gg[:], in0=mconst[:], scalar1=g[:, 0:1])
            stat_ps = ps.tile([COLS, 2], f32, tag="ps_a")
            nc.tensor.matmul(stat_ps[:], lhsT=ones2[:], rhs=gg[:], start=True, stop=True)
            inv_rng = sb.tile([COLS, 1], f32)
            nc.vector.reciprocal(inv_rng[:], stat_ps[:, 0:1])
            outt = sb.tile([COLS, 128], f32)
            nc.vector.tensor_scalar(
                out=outt[:], in0=ps_rm[:], scalar1=stat_ps[:, 1:2],
                scalar2=inv_rng[:, 0:1], op0=mybir.AluOpType.subtract,
                op1=mybir.AluOpType.mult,
            )
            nc.sync.dma_start(
                out=o_d.ap().rearrange("a (c p) -> (a c) p", p=128), in_=outt[:]
            )

    nc.compile()
    return nc


def _get_nc():
    if "nc" not in _CACHE:
        _CACHE["nc"] = _build_bass()
    return _CACHE["nc"]


def _run(in_map, trace=False, **kwargs):
    from concourse.bass_utils import run_bass_kernel_spmd

    nc = _get_nc()
    return run_bass_kernel_spmd(
        nc, [in_map] * NCORES, core_ids=list(range(NCORES)), trace=trace, **kwargs
    )


def kernel(X, weight, noise, sigma):
    in_map = {
        "X": np.ascontiguousarray(X, dtype=np.float32).reshape(1, T),
        "weight": np.ascontiguousarray(weight, dtype=np.float32).reshape(1, D),
        "noise": np.ascontiguousarray(noise, dtype=np.float32).reshape(1, T),
        "sigma": np.ascontiguousarray(sigma, dtype=np.float32).reshape(D),
    }
    res = _run(in_map).results
    return res[0]["out"].reshape(1, T)
